# revision 1
# baseline (speedup 1.0000x reference)
"""Trainium2 Bass kernel for CarlosSelfAttention (B=2, T=2048, C=1024, H=16).

Sharding: tensor-parallel over heads. 8 cores x 2 heads each.
Each core computes q/k/v projections for its 2 heads, RoPE, causal
attention, and a partial out-projection against its 128 columns of Wo.
The host sums the 8 partial outputs (the TP all-reduce) and adds the
output bias plus the (v-bias @ Wo.T) correction term.

All on-chip layouts are "transposed" ([dim, token]) so every matmul
contraction lands on the partition axis:
  xT   [1024, 4096]   (input, replicated)
  qT/kT[128, 4096]    rows = [h0-even dims, h0-odd, h1-even, h1-odd]
  vT   2 x [64, 4096] rows = plain head dims
  S^T  [k-tile 128, q-chunk 512] via PE, exp'd on ScalarE from PSUM
  P@V  col-packed (h0 -> psum rows 0:63, h1 -> 64:127), sums via
       ones-matmul, normalization by reciprocal broadcast.
  out  y_part [4096, 1024] = OT.T @ WoT via PE, DMA'd from PSUM.
"""

import os
import numpy as np

import concourse.bass as bass
import concourse.tile as tile
from concourse import bacc, mybir
from concourse.bass_utils import run_bass_kernel_spmd

F32 = mybir.dt.float32
F32R = mybir.dt.float32r
AF = mybir.ActivationFunctionType

B, T, C, H, HD = 2, 2048, 1024, 16, 64
NCORES = 8
TB = B * T          # 4096
QCH = 512           # q-chunk (moving dim)
NQC = T // QCH      # 4 q-chunks per batch
NKT = T // 128      # 16 k-tiles per batch
NTC = TB // QCH     # 8 t-chunks for the projections
NCT = C // 128      # 8 contraction tiles

_PROG_CACHE: dict = {}


def _emit(tc, mode, dram):
    nc = tc.nc
    from contextlib import ExitStack

    xT, wT, bqk, cosT, sinS, woT, y = (
        dram["xT"], dram["wT"], dram["bqk"], dram["cosT"], dram["sinS"],
        dram["woT"], dram["y"])
    maskT = dram.get("maskT")

    with ExitStack() as ctx:
        constp = ctx.enter_context(tc.tile_pool(name="const", bufs=1))
        pers = ctx.enter_context(tc.tile_pool(name="pers", bufs=1))

        # ---- constants ----
        wsb = constp.tile([128, NCT, 384], F32)
        nc.sync.dma_start(wsb[:].bitcast(F32R),
                          wT[:].rearrange("(a p) m -> p a m", p=128).bitcast(F32R))
        cos_sb = constp.tile([128, T], F32)
        nc.sync.dma_start(cos_sb[:], cosT[:])
        sin_sb = constp.tile([128, T], F32)
        nc.sync.dma_start(sin_sb[:], sinS[:])
        bqk_sb = constp.tile([128, 2], F32)
        nc.sync.dma_start(bqk_sb[:], bqk[:])
        wo_sb = constp.tile([128, C], F32)
        nc.sync.dma_start(wo_sb[:].bitcast(F32R), woT[:].bitcast(F32R))
        ones16 = constp.tile([128, NKT], F32)
        nc.vector.memset(ones16[:], 1.0)
        id64 = constp.tile([64, 64], F32)
        nc.vector.memset(id64[:], 1.0)
        nc.gpsimd.affine_select(
            out=id64[:], in_=id64[:], compare_op=mybir.AluOpType.is_equal,
            fill=0.0, base=0, channel_multiplier=1, pattern=[[-1, 64]])

        # ---- persistent activations ----
        qT = pers.tile([128, TB], F32)
        kT = pers.tile([128, TB], F32)
        vTf = pers.tile([128, TB], F32)
        vT1 = pers.tile([64, TB], F32)
        Vsb = [[pers.tile([128, NKT * (HD + 1)], F32, name=f"Vsb{b}{h}")
                for h in range(2)] for b in range(B)]
        OT = [pers.tile([128, T], F32, name=f"OTb{b}") for b in range(B)]

        def qkv_pair(xp, psqkv, tca, tcb, defer=None):
            """Emit qkv projection for two t-chunks with shared stationaries.
            If defer is a list, append per-(g,ct) units instead of emitting."""
            tsa = slice(tca * QCH, (tca + 1) * QCH)
            tsb = slice(tcb * QCH, (tcb + 1) * QCH)
            xa, xb = [], []
            def load(ct, tci, ts, lst):
                xt = xp.tile([128, QCH], F32, tag="x", name=f"xt{tci}_{ct}")
                nc.sync.dma_start(
                    xt[:].bitcast(F32R),
                    xT[ct * 128:(ct + 1) * 128, ts].bitcast(F32R))
                lst.append(xt)
            def emit_g(g):
                psa = psqkv.tile([128, QCH], F32, tag="ps",
                                 name=f"psq{tca}_{g}")
                psb = psqkv.tile([128, QCH], F32, tag="ps",
                                 name=f"psq{tcb}_{g}")
                def emit_ct(g, ct, psa=psa, psb=psb):
                    w = wsb[:, ct, g * 128:(g + 1) * 128].bitcast(F32R)
                    nc.tensor.matmul(psa[:], w, xa[ct][:].bitcast(F32R),
                                     start=(ct == 0), stop=(ct == NCT - 1))
                    nc.tensor.matmul(psb[:], w, xb[ct][:].bitcast(F32R),
                                     start=(ct == 0), stop=(ct == NCT - 1))
                def evict(g, ps, ts):
                    if g == 0:
                        nc.scalar.activation(qT[:, ts].bitcast(F32R), ps[:],
                                             AF.Identity, bias=bqk_sb[:, 0:1])
                    elif g == 1:
                        nc.scalar.activation(kT[:, ts].bitcast(F32R), ps[:],
                                             AF.Identity, bias=bqk_sb[:, 1:2])
                    else:
                        nc.scalar.activation(vTf[:, ts], ps[:], AF.Copy)
                        nc.sync.dma_start(vT1[:, ts], vTf[64:128, ts])
                for ct in range(NCT):
                    if defer is None:
                        emit_ct(g, ct)
                    else:
                        defer.append(lambda g=g, ct=ct: emit_ct(g, ct))
                if defer is None:
                    evict(g, psa, tsa)
                    evict(g, psb, tsb)
                else:
                    defer.append(lambda g=g, psa=psa, tsa=tsa: evict(g, psa, tsa))
                    defer.append(lambda g=g, psb=psb, tsb=tsb: evict(g, psb, tsb))
            for ct in range(NCT):
                load(ct, tca, tsa, xa)
                load(ct, tcb, tsb, xb)
            for g in range(3):
                emit_g(g)
            return None

        def rope_b(swpp, rtp, zt, b, nm):
            bs = slice(b * T, (b + 1) * T)
            swp = swpp.tile([128, T], F32, tag="swp", name=f"swp{nm}")
            for h in range(2):
                o = h * 64
                nc.sync.dma_start(swp[o:o + 32, :], zt[o + 32:o + 64, bs])
                nc.sync.dma_start(swp[o + 32:o + 64, :], zt[o:o + 32, bs])
            tmp = rtp.tile([128, T], F32, tag="rt", name=f"rt{nm}")
            nc.vector.tensor_mul(tmp[:], swp[:], sin_sb[:])
            nc.vector.tensor_mul(zt[:, bs].bitcast(F32R), zt[:, bs], cos_sb[:])
            nc.vector.tensor_add(zt[:, bs].bitcast(F32R), zt[:, bs], tmp[:])

        def vtrans_b(pstr, b):
            for h, vt in ((0, vTf), (1, vT1)):
                vov = Vsb[b][h][:].rearrange("p (t c) -> p t c", c=HD + 1)
                nc.vector.tensor_copy(vov[:, :, HD:HD + 1].bitcast(F32R),
                                      ones16[:])
                for tt in range(NKT):
                    pst = pstr.tile([128, 64], F32, tag="tr",
                                    name=f"pst{b}{h}{tt}")
                    nc.tensor.transpose(
                        pst[:],
                        vt[0:64, b * T + tt * 128: b * T + (tt + 1) * 128],
                        id64[:])
                    nc.vector.tensor_copy(
                        Vsb[b][h][:, tt * (HD + 1):tt * (HD + 1) + HD]
                        .bitcast(F32R),
                        pst[:])

        def attn_b(pools, b, fillers, qc_done=None):
            pss, pso, ptp, mbp, smol, bcp = pools
            PIPE = 3
            for qc in range(NQC):
                nk = 4 * (qc + 1) if mode == "causal" else NKT
                qs = slice(b * T + qc * QCH, b * T + (qc + 1) * QCH)
                psO0 = pso.tile([65, QCH], F32, tag="o0", name=f"psO0_{b}{qc}")
                psO1 = pso.tile([65, QCH], F32, tag="o1", name=f"psO1_{b}{qc}")
                pts = {}

                def emit_pv(j, nk=nk, psO0=psO0, psO1=psO1, pts=pts):
                    st, sp = (j == 0), (j == nk - 1)
                    pt = pts.pop(j)
                    nc.tensor.matmul(
                        psO0[:],
                        Vsb[b][0][:, j * (HD + 1):(j + 1) * (HD + 1)]
                        .bitcast(F32R),
                        pt[:, 0:QCH].bitcast(F32R), start=st, stop=sp)
                    nc.tensor.matmul(
                        psO1[:],
                        Vsb[b][1][:, j * (HD + 1):(j + 1) * (HD + 1)]
                        .bitcast(F32R),
                        pt[:, QCH:2 * QCH].bitcast(F32R), start=st, stop=sp)

                for kt in range(nk):
                    ks = slice(b * T + kt * 128, b * T + (kt + 1) * 128)
                    psS = pss.tile([128, 2 * QCH], F32, tag="s",
                                   name=f"psS{b}{qc}{kt}")
                    nc.tensor.matmul(psS[:, 0:QCH],
                                     kT[0:64, ks].bitcast(F32R),
                                     qT[0:64, qs].bitcast(F32R),
                                     start=True, stop=True)
                    nc.tensor.matmul(psS[:, QCH:2 * QCH],
                                     kT[64:128, ks].bitcast(F32R),
                                     qT[64:128, qs].bitcast(F32R),
                                     start=True, stop=True)
                    pt = ptp.tile([128, 2 * QCH], F32, tag="pt",
                                  name=f"pt{b}{qc}{kt}")
                    nc.scalar.activation(pt[:].bitcast(F32R), psS[:], AF.Exp)
                    if mode == "causal" and kt >= 4 * qc:
                        base = qc * QCH - kt * 128
                        ptv = pt[:].rearrange("p (h q) -> p h q", q=QCH)
                        nc.gpsimd.affine_select(
                            out=ptv.bitcast(F32R), in_=ptv.bitcast(F32R),
                            compare_op=mybir.AluOpType.is_ge,
                            fill=0.0, base=base, channel_multiplier=-1,
                            pattern=[[0, 2], [1, QCH]])
                    elif mode == "bias":
                        mt = mbp.tile([128, QCH], F32, tag="mb",
                                      name=f"mt{b}{qc}{kt}")
                        nc.sync.dma_start(
                            mt[:], maskT[kt * 128:(kt + 1) * 128,
                                         qc * QCH:(qc + 1) * QCH])
                        nc.vector.tensor_mul(pt[:, 0:QCH].bitcast(F32R),
                                             pt[:, 0:QCH], mt[:])
                        nc.vector.tensor_mul(pt[:, QCH:2 * QCH].bitcast(F32R),
                                             pt[:, QCH:2 * QCH], mt[:])
                    pts[kt] = pt
                    if fillers:
                        fillers.popleft()()
                    if kt >= PIPE:
                        emit_pv(kt - PIPE)
                for j in range(max(0, nk - PIPE), nk):
                    emit_pv(j)

                # normalize + evict; sum(exp) in row 64 of psO*
                oqs = slice(qc * QCH, (qc + 1) * QCH)
                for h, psO in ((0, psO0), (1, psO1)):
                    nm = f"{b}{qc}{h}"
                    rw = smol.tile([65, QCH], F32, tag="rw", name=f"rw{nm}")
                    nc.scalar.activation(rw[64:65, :], psO[64:65, :], AF.Copy)
                    rz = smol.tile([1, QCH], F32, tag="rz", name=f"rz{nm}")
                    nc.sync.dma_start(rz[:], rw[64:65, :])
                    rr = smol.tile([1, QCH], F32, tag="rr", name=f"rr{nm}")
                    nc.vector.reciprocal_approx_fast(rr[:], rz[:])
                    bc = bcp.tile([128, QCH], F32, tag="bc", name=f"bc{nm}")
                    nc.gpsimd.partition_broadcast(bc[:], rr[:])
                    if h == 0:
                        nc.vector.tensor_mul(OT[b][0:64, oqs].bitcast(F32R),
                                             psO[0:64, :], bc[0:64, :])
                    else:
                        otmp = bcp.tile([64, QCH], F32, tag="otmp",
                                        name=f"otmp{nm}")
                        nc.vector.tensor_mul(otmp[:], psO[0:64, :],
                                             bc[0:64, :])
                        nc.sync.dma_start(OT[b][64:128, oqs].bitcast(F32R),
                                          otmp[:].bitcast(F32R))
                if qc_done is not None:
                    qc_done(qc)

        def proj_unit_fn(psy, ybp, b):
            def unit_for(tt):
                def unit(tt=tt, b=b):
                    for ncol in range(2):
                        nm = f"{b}{tt}{ncol}"
                        ps = psy.tile([128, QCH], F32, tag="y",
                                      name=f"psy{nm}")
                        nc.tensor.matmul(
                            ps[:],
                            OT[b][:, tt * 128:(tt + 1) * 128].bitcast(F32R),
                            wo_sb[:, ncol * QCH:(ncol + 1) * QCH]
                            .bitcast(F32R),
                            start=True, stop=True)
                        yb = ybp.tile([128, QCH], F32, tag="yb",
                                      name=f"yb{nm}")
                        nc.vector.tensor_copy(yb[:], ps[:])
                        nc.sync.dma_start(
                            y[b * T + tt * 128: b * T + (tt + 1) * 128,
                              ncol * QCH:(ncol + 1) * QCH], yb[:])
                return unit
            return unit_for

        # ---- phases, per batch ----
        with tc.tile_pool(name="xp", bufs=16) as xp, \
             tc.tile_pool(name="psqkv", bufs=4, space="PSUM") as psqkv, \
             tc.tile_pool(name="pstr", bufs=2, space="PSUM") as pstr, \
             tc.tile_pool(name="swp", bufs=2) as swpp, \
             tc.tile_pool(name="rtmp", bufs=2) as rtp:
            qkv_pair(xp, psqkv, 0, 1)
            qkv_pair(xp, psqkv, 2, 3)
            rope_b(swpp, rtp, qT, 0, "q0")
            rope_b(swpp, rtp, kT, 0, "k0")
            vtrans_b(pstr, 0)
            qkv_pair(xp, psqkv, 4, 5)
            qkv_pair(xp, psqkv, 6, 7)
            rope_b(swpp, rtp, qT, 1, "q1")
            rope_b(swpp, rtp, kT, 1, "k1")
            vtrans_b(pstr, 1)

        from collections import deque
        for b in range(B):
            with tc.tile_pool(name="pss", bufs=2, space="PSUM") as pss, \
                 tc.tile_pool(name="pso", bufs=2, space="PSUM") as pso, \
                 tc.tile_pool(name="ptp", bufs=5) as ptp, \
                 tc.tile_pool(name="mbp", bufs=4) as mbp, \
                 tc.tile_pool(name="smol", bufs=4) as smol, \
                 tc.tile_pool(name="bcp", bufs=4) as bcp:
                attn_b((pss, pso, ptp, mbp, smol, bcp), b, deque())
            with tc.tile_pool(name="psy", bufs=4, space="PSUM") as psy, \
                 tc.tile_pool(name="ybp", bufs=4) as ybp:
                unit = proj_unit_fn(psy, ybp, b)
                for tt in range(NKT):
                    unit(tt)()


def _build_program(mode):
    if mode in _PROG_CACHE:
        return _PROG_CACHE[mode]
    nc = bacc.Bacc("TRN2", target_bir_lowering=False, debug=False,
                   num_devices=NCORES)
    dram = {
        "xT": nc.dram_tensor("xT", [C, TB], F32, kind="ExternalInput").ap(),
        "wT": nc.dram_tensor("wT", [C, 384], F32, kind="ExternalInput").ap(),
        "bqk": nc.dram_tensor("bqk", [128, 2], F32, kind="ExternalInput").ap(),
        "cosT": nc.dram_tensor("cosT", [128, T], F32, kind="ExternalInput").ap(),
        "sinS": nc.dram_tensor("sinS", [128, T], F32, kind="ExternalInput").ap(),
        "woT": nc.dram_tensor("woT", [128, C], F32, kind="ExternalInput").ap(),
        "y": nc.dram_tensor("y", [TB, C], F32, kind="ExternalOutput").ap(),
    }
    if mode == "bias":
        dram["maskT"] = nc.dram_tensor("maskT", [T, T], F32,
                                       kind="ExternalInput").ap()
    with tile.TileContext(nc) as tc:
        _emit(tc, mode, dram)
    nc.compile()
    _PROG_CACHE[mode] = (nc, dram)
    return nc, dram


def _rope_tables():
    inv_freq = 1.0 / (10000.0 ** (np.arange(0, HD, 2, dtype=np.float64) / HD))
    freqs = np.arange(T, dtype=np.float64)[:, None] * inv_freq[None, :]
    cos = np.concatenate([np.cos(freqs), np.cos(freqs)], axis=-1)  # [T, 64]
    sin = np.concatenate([np.sin(freqs), np.sin(freqs)], axis=-1)
    cE = cos[:, 0::2].T  # [32, T] rows i -> dim 2i
    cO = cos[:, 1::2].T
    sE = sin[:, 0::2].T
    sO = sin[:, 1::2].T
    cosT = np.concatenate([cE, cO, cE, cO], axis=0).astype(np.float32)
    sinS = np.concatenate([-sE, sO, -sE, sO], axis=0).astype(np.float32)
    return np.ascontiguousarray(cosT), np.ascontiguousarray(sinS)


def kernel(x, mask, Wqkv, bqkv, Wo, bo):
    x = np.asarray(x, dtype=np.float32)
    mask = np.asarray(mask)
    Wqkv = np.asarray(Wqkv, dtype=np.float32)
    bqkv = np.asarray(bqkv, dtype=np.float32)
    Wo = np.asarray(Wo, dtype=np.float32)
    bo = np.asarray(bo, dtype=np.float32)

    mb = mask.reshape(T, T)
    if np.array_equal(mb != 0, np.tril(np.ones((T, T), dtype=bool))):
        mode = "causal"
    elif np.all(mb != 0):
        mode = "dense"
    else:
        mode = "bias"

    nc, dram = _build_program(mode)

    xTn = np.ascontiguousarray(x.reshape(TB, C).T)
    cosT, sinS = _rope_tables()
    scale = 1.0 / np.sqrt(np.float32(HD))

    evens = np.arange(0, HD, 2)
    odds = evens + 1

    in_maps = []
    for c in range(NCORES):
        h0, h1 = 2 * c, 2 * c + 1
        qrows = np.concatenate([h0 * HD + evens, h0 * HD + odds,
                                h1 * HD + evens, h1 * HD + odds])
        krows = C + qrows
        vrows = np.concatenate([2 * C + h0 * HD + np.arange(HD),
                                2 * C + h1 * HD + np.arange(HD)])
        wq = Wqkv[qrows, :] * scale
        wk = Wqkv[krows, :]
        wv = Wqkv[vrows, :]
        wT = np.ascontiguousarray(np.concatenate([wq, wk, wv], axis=0).T)
        bqk = np.stack([bqkv[qrows] * scale, bqkv[krows]], axis=1)
        woT = np.ascontiguousarray(Wo[:, 128 * c:128 * (c + 1)].T)
        im = {
            "xT": xTn, "wT": wT,
            "bqk": np.ascontiguousarray(bqk, dtype=np.float32),
            "cosT": cosT, "sinS": sinS, "woT": woT,
        }
        if mode == "bias":
            im["maskT"] = np.ascontiguousarray(
                (mb != 0).astype(np.float32).T)
        in_maps.append(im)

    res = run_bass_kernel_spmd(nc, in_maps, core_ids=list(range(NCORES)))
    y = np.zeros((TB, C), dtype=np.float32)
    for c in range(NCORES):
        y += res.results[c]["y"]
    bv = bqkv[2 * C:3 * C]
    y += (bo + bv @ Wo.T)[None, :]
    return y.reshape(B, T, C)



# revision 7
# speedup vs baseline: 1.2724x; 1.2724x over previous
"""Trainium2 Bass kernel for CarlosSelfAttention (B=2, T=2048, C=1024, H=16).

Sharding: tensor-parallel over heads. 8 cores x 2 heads each.
Each core computes q/k/v projections for its 2 heads, RoPE, causal
attention, and a partial out-projection against its 128 columns of Wo.
The host sums the 8 partial outputs (the TP all-reduce) and adds the
output bias plus the (v-bias @ Wo.T) correction term.

v1 redesign vs baseline:
  - bf16 everywhere on-chip (PSUM stays fp32); x/weights/rope tables are
    host-cast to bf16; y partials returned as bf16 (summed on host in f32).
  - Single-pass pipelined emission: qkv(b0) -> [attn(b0) with qkv(b1) +
    rope(b1) + vtrans(b1) as PE/vector fillers] -> [attn(b1) with proj
    fillers] -> proj tail. Keeps TensorE dense so the HAM clock stays at
    2.4 GHz (baseline ran ~70% of the kernel at 1.2 GHz).
  - All of x preloaded into SBUF via 8 chunked DMAs (1 MB each).
  - PSUM plan (8 banks): psS 2x[128,1024] (4) + psO 1x[65,1024] (2) +
    filler/proj/transpose pool 2x[128,512] (2).
  - Softmax denominator via ones-column in the PV stationary (row 64 of
    psO); normalization = row-copy -> partition-shift DMA -> gpsimd
    broadcast -> DVE divide.
"""

import numpy as np

import concourse.bass as bass
import concourse.tile as tile
from concourse import bacc, mybir
from concourse.bass_utils import run_bass_kernel_spmd

F32 = mybir.dt.float32
BF16 = mybir.dt.bfloat16
AF = mybir.ActivationFunctionType
ALU = mybir.AluOpType

B, T, C, H, HD = 2, 2048, 1024, 16, 64
NCORES = 8
TB = B * T          # 4096
QCH = 512           # q-chunk (moving dim)
NQC = T // QCH      # 4 q-chunks per batch
NKT = T // 128      # 16 k-tiles per batch
NCT = C // 128      # 8 contraction tiles
PIPE = 3            # exp->PV pipeline lag (in kt units)
USE_DIV = False     # DVE divide is not a valid TT op on trn2; use recip+mul

_PROG_CACHE: dict = {}


def _emit(tc, mode, zero_bias, dram):
    nc = tc.nc
    from contextlib import ExitStack
    from collections import deque

    xT, wT, cosT, sinS, woT, y = (
        dram["xT"], dram["wT"], dram["cosT"], dram["sinS"], dram["woT"],
        dram["y"])
    bqk = dram.get("bqk")
    maskT = dram.get("maskT")

    with ExitStack() as ctx:
        constp = ctx.enter_context(tc.tile_pool(name="const", bufs=1))
        pers = ctx.enter_context(tc.tile_pool(name="pers", bufs=1))

        # ---- persistent tiles ----
        xsb = pers.tile([128, NCT, TB], BF16)     # all of x, [c128, ct, tok]
        qT = pers.tile([128, TB], BF16)
        kT = pers.tile([128, TB], BF16)
        vTf = pers.tile([128, TB], BF16)
        Vsb = [[pers.tile([128, NKT * (HD + 1)], BF16, name=f"Vsb{b}{h}")
                for h in range(2)] for b in range(B)]
        OT = [pers.tile([128, T], BF16, name=f"OTb{b}") for b in range(B)]

        wsb = constp.tile([128, NCT, 384], BF16)
        cos_sb = constp.tile([128, T], BF16)
        sin_sb = constp.tile([128, T], BF16)
        wo_sb = constp.tile([128, C], BF16)
        id2 = constp.tile([128, 64], BF16)  # [eye(64); eye(64)] host-fed
        if not zero_bias:
            bqk_sb = constp.tile([128, 2], F32)

        nc.sync.dma_start(id2[:], dram["id2"][:])
        # ones column in each Vsb 65-block (the softmax-denominator row)
        for b in range(B):
            for h in range(2):
                vov = Vsb[b][h][:].rearrange("p (t c) -> p t c", c=HD + 1)
                nc.vector.memset(vov[:, :, HD:HD + 1], 1.0)

        # ---- DMA issue order: wsb, x0-1, cos/sin, x2-7, wo ----
        nc.sync.dma_start(wsb[:], wT[:].rearrange("(a p) m -> p a m", p=128))
        if not zero_bias:
            nc.sync.dma_start(bqk_sb[:], bqk[:])

        def load_x(c):
            ts = slice(c * QCH, (c + 1) * QCH)
            nc.sync.dma_start(
                xsb[:, :, ts], xT[:, ts].rearrange("(a p) m -> p a m", p=128))
        load_x(0)
        load_x(1)
        nc.sync.dma_start(cos_sb[:], cosT[:])
        nc.sync.dma_start(sin_sb[:], sinS[:])
        for c in range(2, 8):
            load_x(c)
        nc.sync.dma_start(wo_sb[:], woT[:])

        # ================= emission helpers =================
        evict_flip = [0]

        def qkv_unit(pool, c, g):
            """One projection group for one 512-token chunk: 8 accumulating
            matmuls + one psum->sbuf eviction (with cast to bf16)."""
            ts = slice(c * QCH, (c + 1) * QCH)
            ps = pool.tile([128, QCH], F32, tag="f", name=f"psq{c}{g}")
            for ct in range(NCT):
                nc.tensor.matmul(ps[:], wsb[:, ct, g * 128:(g + 1) * 128],
                                 xsb[:, ct, ts],
                                 start=(ct == 0), stop=(ct == NCT - 1))
            dst = (qT, kT, vTf)[g]
            if zero_bias:
                nc.vector.tensor_copy(dst[:, ts], ps[:])
            else:
                if g == 2:
                    nc.scalar.activation(dst[:, ts], ps[:], AF.Copy)
                else:
                    nc.scalar.activation(dst[:, ts], ps[:], AF.Identity,
                                         bias=bqk_sb[:, g:g + 1])

        def vtrans_unit(pool, b, h, c):
            """Transpose V for the 4 k-tiles of one chunk into Vsb."""
            pst = pool.tile([128, 4 * HD], BF16, tag="f", name=f"pst{b}{h}{c}")
            t0 = 4 * (c % 4)
            for j in range(4):
                tt = t0 + j
                nc.tensor.transpose(
                    pst[:, j * HD:(j + 1) * HD],
                    vTf[h * HD:(h + 1) * HD,
                        b * T + tt * 128: b * T + (tt + 1) * 128],
                    id2[h * HD:(h + 1) * HD, :])
            vov = Vsb[b][h][:].rearrange("p (t c) -> p t c", c=HD + 1)
            nc.vector.tensor_copy(
                vov[:, t0:t0 + 4, 0:HD],
                pst[:].rearrange("p (t c) -> p t c", c=HD))

        def rope_half(swpp, rtp, zt, b, nm):
            bs = slice(b * T, (b + 1) * T)
            swp = swpp.tile([128, T], BF16, tag="swp", name=f"swp{nm}")
            for h in range(2):
                o = h * 64
                nc.sync.dma_start(swp[o:o + 32, :], zt[o + 32:o + 64, bs])
                nc.sync.dma_start(swp[o + 32:o + 64, :], zt[o:o + 32, bs])
            tmp = rtp.tile([128, T], BF16, tag="rt", name=f"rt{nm}")
            nc.vector.tensor_mul(tmp[:], swp[:], sin_sb[:])
            nc.vector.tensor_mul(zt[:, bs], zt[:, bs], cos_sb[:])
            nc.vector.tensor_add(zt[:, bs], zt[:, bs], tmp[:])

        def proj_unit(pool, ybp, b, tt):
            """Out-projection for one 128-token tile: 2 matmuls + eviction +
            y DMA (bf16)."""
            for ncol in range(2):
                ps = pool.tile([128, QCH], F32, tag="f",
                               name=f"psy{b}{tt}{ncol}")
                nc.tensor.matmul(
                    ps[:], OT[b][:, tt * 128:(tt + 1) * 128],
                    wo_sb[:, ncol * QCH:(ncol + 1) * QCH],
                    start=True, stop=True)
                yb = ybp.tile([128, QCH], BF16, tag="yb",
                              name=f"yb{b}{tt}{ncol}")
                if evict_flip[0] % 2 == 0:
                    nc.vector.tensor_copy(yb[:], ps[:])
                else:
                    nc.scalar.activation(yb[:], ps[:], AF.Copy)
                evict_flip[0] += 1
                nc.sync.dma_start(
                    y[b * T + tt * 128: b * T + (tt + 1) * 128,
                      ncol * QCH:(ncol + 1) * QCH], yb[:])

        # ================= attention =================
        def attn_b(pools, b, fillers, post_qc=None):
            pss, pso, ptp, mbp, smol, bcp = pools
            ucount = [0]
            for qc in range(NQC):
                nk = 4 * (qc + 1) if mode == "causal" else NKT
                qs = slice(b * T + qc * QCH, b * T + (qc + 1) * QCH)
                psO = pso.tile([65, 2 * QCH], F32, tag="o", name=f"psO{b}{qc}")
                pts = {}

                def emit_pv(j, nk=nk, psO=psO, pts=pts):
                    st, sp = (j == 0), (j == nk - 1)
                    pt = pts.pop(j)
                    nc.tensor.matmul(
                        psO[:, 0:QCH],
                        Vsb[b][0][:, j * (HD + 1):(j + 1) * (HD + 1)],
                        pt[:, 0:QCH], start=st, stop=sp)
                    nc.tensor.matmul(
                        psO[:, QCH:2 * QCH],
                        Vsb[b][1][:, j * (HD + 1):(j + 1) * (HD + 1)],
                        pt[:, QCH:2 * QCH], start=st, stop=sp)

                for kt in range(nk):
                    ks = slice(b * T + kt * 128, b * T + (kt + 1) * 128)
                    psS = pss.tile([128, 2 * QCH], F32, tag="s",
                                   name=f"psS{b}{qc}{kt}")
                    nc.tensor.matmul(psS[:, 0:QCH], kT[0:64, ks],
                                     qT[0:64, qs], start=True, stop=True)
                    nc.tensor.matmul(psS[:, QCH:2 * QCH], kT[64:128, ks],
                                     qT[64:128, qs], start=True, stop=True)
                    pt = ptp.tile([128, 2 * QCH], BF16, tag="pt",
                                  name=f"pt{b}{qc}{kt}")
                    nc.scalar.activation(pt[:], psS[:], AF.Exp)
                    if mode == "causal" and kt >= 4 * qc:
                        base = qc * QCH - kt * 128
                        ptv = pt[:].rearrange("p (h q) -> p h q", q=QCH)
                        nc.gpsimd.affine_select(
                            out=ptv, in_=ptv, compare_op=ALU.is_ge,
                            fill=0.0, base=base, channel_multiplier=-1,
                            pattern=[[0, 2], [1, QCH]])
                    elif mode == "bias":
                        mt = mbp.tile([128, QCH], BF16, tag="mb",
                                      name=f"mt{b}{qc}{kt}")
                        nc.sync.dma_start(
                            mt[:], maskT[kt * 128:(kt + 1) * 128,
                                         qc * QCH:(qc + 1) * QCH])
                        nc.vector.tensor_mul(pt[:, 0:QCH], pt[:, 0:QCH],
                                             mt[:])
                        nc.vector.tensor_mul(pt[:, QCH:2 * QCH],
                                             pt[:, QCH:2 * QCH], mt[:])
                    pts[kt] = pt
                    # filler work to keep the PE warm while ScalarE exps
                    ucount[0] += 1
                    if fillers and ucount[0] >= 2:
                        fillers.popleft()()
                    if kt >= PIPE:
                        emit_pv(kt - PIPE)
                for j in range(max(0, nk - PIPE), nk):
                    emit_pv(j)

                # ---- normalize + evict: sum(exp) is row 64 of psO ----
                oqs = slice(qc * QCH, (qc + 1) * QCH)
                nm = f"{b}{qc}"
                rw = smol.tile([65, 2 * QCH], F32, tag="rw", name=f"rw{nm}")
                nc.vector.tensor_copy(rw[64:65, :], psO[64:65, :])
                rz = smol.tile([1, 2 * QCH], F32, tag="rz", name=f"rz{nm}")
                nc.sync.dma_start(rz[:], rw[64:65, :])
                bc = bcp.tile([128, 2 * QCH], F32, tag="bc", name=f"bc{nm}")
                nc.gpsimd.partition_broadcast(bc[:], rz[:])
                otmp = bcp.tile([64, QCH], BF16, tag="otmp", name=f"ot{nm}")
                if USE_DIV:
                    nc.vector.tensor_tensor(OT[b][0:64, oqs],
                                            psO[0:64, 0:QCH],
                                            bc[0:64, 0:QCH], op=ALU.divide)
                    nc.vector.tensor_tensor(otmp[:], psO[0:64, QCH:2 * QCH],
                                            bc[0:64, QCH:2 * QCH],
                                            op=ALU.divide)
                else:
                    bcr = bcp.tile([128, 2 * QCH], F32, tag="bcr",
                                   name=f"bcr{nm}")
                    nc.vector.reciprocal_approx_fast(bcr[:], bc[:])
                    nc.vector.tensor_mul(OT[b][0:64, oqs], psO[0:64, 0:QCH],
                                         bcr[0:64, 0:QCH])
                    nc.vector.tensor_mul(otmp[:], psO[0:64, QCH:2 * QCH],
                                         bcr[0:64, QCH:2 * QCH])
                nc.sync.dma_start(OT[b][64:128, oqs], otmp[:])
                if post_qc is not None:
                    post_qc(qc)

        # ================= phases =================
        # Phase A: qkv + vtrans for batch 0 (dedicated psum pool), rope(b0)
        swpp = ctx.enter_context(tc.tile_pool(name="swp", bufs=2))
        rtp = ctx.enter_context(tc.tile_pool(name="rtmp", bufs=2))
        with tc.tile_pool(name="psA", bufs=4, space="PSUM") as psA:
            for c in range(4):
                for g in (2, 0, 1):
                    qkv_unit(psA, c, g)
                for h in range(2):
                    vtrans_unit(psA, 0, h, c)
            rope_half(swpp, rtp, qT, 0, "q0")
            rope_half(swpp, rtp, kT, 0, "k0")

        # Phase B: attention with fillers
        with tc.tile_pool(name="pss", bufs=2, space="PSUM") as pss, \
             tc.tile_pool(name="pso", bufs=1, space="PSUM") as pso, \
             tc.tile_pool(name="psf", bufs=2, space="PSUM") as psf, \
             tc.tile_pool(name="ptp", bufs=PIPE + 2) as ptp, \
             tc.tile_pool(name="mbp", bufs=4) as mbp, \
             tc.tile_pool(name="smol", bufs=2) as smol, \
             tc.tile_pool(name="bcp", bufs=2) as bcp, \
             tc.tile_pool(name="ybp", bufs=4) as ybp:
            pools = (pss, pso, ptp, mbp, smol, bcp)

            fillers = deque()
            for c in range(4, 8):
                for g in (2, 0, 1):
                    fillers.append(lambda c=c, g=g: qkv_unit(psf, c, g))
                for h in range(2):
                    fillers.append(
                        lambda h=h, c=c: vtrans_unit(psf, 1, h, c))
            fillers.append(lambda: rope_half(swpp, rtp, qT, 1, "q1"))
            fillers.append(lambda: rope_half(swpp, rtp, kT, 1, "k1"))

            attn_b(pools, 0, fillers)
            while fillers:
                fillers.popleft()()

            # proj(b0) interleaved into attn(b1); proj(b1) per-qc after norm
            fillers2 = deque()
            for tt in range(NKT):
                fillers2.append(lambda tt=tt: proj_unit(psf, ybp, 0, tt))

            def post_qc_b1(qc):
                for tt in range(4 * qc, 4 * qc + 4):
                    fillers2.append(
                        lambda tt=tt: proj_unit(psf, ybp, 1, tt))

            attn_b(pools, 1, fillers2, post_qc=post_qc_b1)
            while fillers2:
                fillers2.popleft()()


def _build_program(mode, zero_bias):
    key = (mode, zero_bias)
    if key in _PROG_CACHE:
        return _PROG_CACHE[key]
    nc = bacc.Bacc("TRN2", target_bir_lowering=False, debug=False,
                   num_devices=NCORES)
    dram = {
        "xT": nc.dram_tensor("xT", [C, TB], BF16, kind="ExternalInput").ap(),
        "wT": nc.dram_tensor("wT", [C, 384], BF16, kind="ExternalInput").ap(),
        "cosT": nc.dram_tensor("cosT", [128, T], BF16,
                               kind="ExternalInput").ap(),
        "sinS": nc.dram_tensor("sinS", [128, T], BF16,
                               kind="ExternalInput").ap(),
        "woT": nc.dram_tensor("woT", [128, C], BF16,
                              kind="ExternalInput").ap(),
        "id2": nc.dram_tensor("id2", [128, 64], BF16,
                              kind="ExternalInput").ap(),
        "y": nc.dram_tensor("y", [TB, C], BF16, kind="ExternalOutput").ap(),
    }
    if not zero_bias:
        dram["bqk"] = nc.dram_tensor("bqk", [128, 2], F32,
                                     kind="ExternalInput").ap()
    if mode == "bias":
        dram["maskT"] = nc.dram_tensor("maskT", [T, T], BF16,
                                       kind="ExternalInput").ap()
    with tile.TileContext(nc) as tc:
        _emit(tc, mode, zero_bias, dram)
    nc.compile()
    _PROG_CACHE[key] = (nc, dram)
    return nc, dram


def _rope_tables():
    inv_freq = 1.0 / (10000.0 ** (np.arange(0, HD, 2, dtype=np.float64) / HD))
    freqs = np.arange(T, dtype=np.float64)[:, None] * inv_freq[None, :]
    cos = np.concatenate([np.cos(freqs), np.cos(freqs)], axis=-1)  # [T, 64]
    sin = np.concatenate([np.sin(freqs), np.sin(freqs)], axis=-1)
    cE = cos[:, 0::2].T  # [32, T] rows i -> dim 2i
    cO = cos[:, 1::2].T
    sE = sin[:, 0::2].T
    sO = sin[:, 1::2].T
    cosT = np.concatenate([cE, cO, cE, cO], axis=0)
    sinS = np.concatenate([-sE, sO, -sE, sO], axis=0)
    return cosT, sinS


def _prepare(x, mask, Wqkv, bqkv, Wo, bo):
    """Host-side prep shared by kernel() and test harness profiling."""
    from ml_dtypes import bfloat16

    x = np.asarray(x, dtype=np.float32)
    mask = np.asarray(mask)
    Wqkv = np.asarray(Wqkv, dtype=np.float32)
    bqkv = np.asarray(bqkv, dtype=np.float32)
    Wo = np.asarray(Wo, dtype=np.float32)

    mb = mask.reshape(T, T)
    if np.array_equal(mb != 0, np.tril(np.ones((T, T), dtype=bool))):
        mode = "causal"
    elif np.all(mb != 0):
        mode = "dense"
    else:
        mode = "bias"
    zero_bias = bool(np.all(bqkv == 0.0))

    xTn = np.ascontiguousarray(x.reshape(TB, C).T).astype(bfloat16)
    cosT, sinS = _rope_tables()
    cosT = np.ascontiguousarray(cosT).astype(bfloat16)
    sinS = np.ascontiguousarray(sinS).astype(bfloat16)
    scale = 1.0 / np.sqrt(np.float64(HD))

    evens = np.arange(0, HD, 2)
    odds = evens + 1

    in_maps = []
    for c in range(NCORES):
        h0, h1 = 2 * c, 2 * c + 1
        qrows = np.concatenate([h0 * HD + evens, h0 * HD + odds,
                                h1 * HD + evens, h1 * HD + odds])
        krows = C + qrows
        vrows = np.concatenate([2 * C + h0 * HD + np.arange(HD),
                                2 * C + h1 * HD + np.arange(HD)])
        wq = Wqkv[qrows, :] * scale
        wk = Wqkv[krows, :]
        wv = Wqkv[vrows, :]
        wT = np.ascontiguousarray(
            np.concatenate([wq, wk, wv], axis=0).T).astype(bfloat16)
        woT = np.ascontiguousarray(Wo[:, 128 * c:128 * (c + 1)].T
                                   ).astype(bfloat16)
        id2 = np.concatenate([np.eye(64), np.eye(64)], axis=0).astype(bfloat16)
        im = {"xT": xTn, "wT": wT, "cosT": cosT, "sinS": sinS, "woT": woT,
              "id2": id2}
        if not zero_bias:
            bqk = np.stack([bqkv[qrows] * scale, bqkv[krows]], axis=1)
            im["bqk"] = np.ascontiguousarray(bqk, dtype=np.float32)
        if mode == "bias":
            im["maskT"] = np.ascontiguousarray(
                (mb != 0).astype(np.float32).T).astype(bfloat16)
        in_maps.append(im)
    return mode, zero_bias, in_maps


def kernel(x, mask, Wqkv, bqkv, Wo, bo):
    bqkv = np.asarray(bqkv, dtype=np.float32)
    Wo = np.asarray(Wo, dtype=np.float32)
    bo = np.asarray(bo, dtype=np.float32)

    mode, zero_bias, in_maps = _prepare(x, mask, Wqkv, bqkv, Wo, bo)
    nc, dram = _build_program(mode, zero_bias)

    res = run_bass_kernel_spmd(nc, in_maps, core_ids=list(range(NCORES)))
    y = np.zeros((TB, C), dtype=np.float32)
    for c in range(NCORES):
        y += np.asarray(res.results[c]["y"], dtype=np.float32)
    bv = bqkv[2 * C:3 * C]
    y += (bo + bv @ Wo.T)[None, :]
    return y.reshape(B, T, C)


# revision 19
# speedup vs baseline: 1.3196x; 1.0371x over previous
"""Trainium2 Bass kernel for CarlosSelfAttention (B=2, T=2048, C=1024, H=16).

Sharding: tensor-parallel over heads. 8 cores x 2 heads each.
Each core computes q/k/v projections for its 2 heads, RoPE, causal
attention, and a partial out-projection against its 128 columns of Wo.
The host sums the 8 partial outputs (the TP all-reduce) and adds the
output bias plus the (v-bias @ Wo.T) correction term.

v1 redesign vs baseline:
  - bf16 everywhere on-chip (PSUM stays fp32); x/weights/rope tables are
    host-cast to bf16; y partials returned as bf16 (summed on host in f32).
  - Single-pass pipelined emission: qkv(b0) -> [attn(b0) with qkv(b1) +
    rope(b1) + vtrans(b1) as PE/vector fillers] -> [attn(b1) with proj
    fillers] -> proj tail. Keeps TensorE dense so the HAM clock stays at
    2.4 GHz (baseline ran ~70% of the kernel at 1.2 GHz).
  - All of x preloaded into SBUF via 8 chunked DMAs (1 MB each).
  - PSUM plan (8 banks): psS 2x[128,1024] (4) + psO 1x[65,1024] (2) +
    filler/proj/transpose pool 2x[128,512] (2).
  - Softmax denominator via ones-column in the PV stationary (row 64 of
    psO); normalization = row-copy -> partition-shift DMA -> gpsimd
    broadcast -> DVE divide.
"""

import numpy as np

import concourse.bass as bass
import concourse.tile as tile
from concourse import bacc, mybir
from concourse.bass_utils import run_bass_kernel_spmd

F32 = mybir.dt.float32
BF16 = mybir.dt.bfloat16
AF = mybir.ActivationFunctionType
ALU = mybir.AluOpType

B, T, C, H, HD = 2, 2048, 1024, 16, 64
NCORES = 8
TB = B * T          # 4096
QCH = 512           # q-chunk (moving dim)
NQC = T // QCH      # 4 q-chunks per batch
NKT = T // 128      # 16 k-tiles per batch
NCT = C // 128      # 8 contraction tiles
PIPE = 7            # exp->PV pipeline lag (in kt units); also hides the
                    # per-qc normalization latency behind the exp stream
USE_DIV = False     # DVE divide is not a valid TT op on trn2; use recip+mul
DIRECT_BCAST = False  # p-bcast ignores AP base partition; needs the p64->p0 DMA

_PROG_CACHE: dict = {}


def _emit(tc, mode, zero_bias, dram):
    nc = tc.nc
    from contextlib import ExitStack
    from collections import deque

    xT, wT, cosT, sinS, woT, y = (
        dram["xT"], dram["wT"], dram["cosT"], dram["sinS"], dram["woT"],
        dram["y"])
    bqk = dram.get("bqk")
    maskT = dram.get("maskT")

    with ExitStack() as ctx:
        constp = ctx.enter_context(tc.tile_pool(name="const", bufs=1))
        pers = ctx.enter_context(tc.tile_pool(name="pers", bufs=1))

        # ---- persistent tiles ----
        xsb = pers.tile([128, NCT, TB], BF16)     # all of x, [c128, ct, tok]
        qT = pers.tile([128, TB], BF16)
        kT = pers.tile([128, TB], BF16)
        vTf = pers.tile([128, TB], BF16)
        Vsb = [[pers.tile([128, NKT * (HD + 1)], BF16, name=f"Vsb{b}{h}")
                for h in range(2)] for b in range(B)]
        OT = [pers.tile([128, T], BF16, name=f"OTb{b}") for b in range(B)]

        wsb = constp.tile([128, NCT, 384], BF16)
        cos_sb = constp.tile([128, T], BF16)
        sin_sb = constp.tile([128, T], BF16)
        wo_sb = constp.tile([128, C], BF16)
        id2 = constp.tile([128, 64], BF16)  # [eye(64); eye(64)] host-fed
        if not zero_bias:
            bqk_sb = constp.tile([128, 2], F32)

        # ones column in each Vsb 65-block (the softmax-denominator row)
        for b in range(B):
            for h in range(2):
                vov = Vsb[b][h][:].rearrange("p (t c) -> p t c", c=HD + 1)
                nc.vector.memset(vov[:, :, HD:HD + 1], 1.0)

        # ---- DMA issue order tuned so the first qkv matmul starts ASAP ----
        def load_w(g):
            gs = slice(g * 128, (g + 1) * 128)
            nc.sync.dma_start(
                wsb[:, :, gs],
                wT[:, gs].rearrange("(a p) m -> p a m", p=128))

        def load_x(c, half=None):
            ts = slice(c * QCH, (c + 1) * QCH)
            if half is None:
                nc.sync.dma_start(
                    xsb[:, :, ts],
                    xT[:, ts].rearrange("(a p) m -> p a m", p=128))
            else:
                cs = slice(half * 4, half * 4 + 4)
                rs = slice(half * 512, half * 512 + 512)
                nc.sync.dma_start(
                    xsb[:, cs, ts],
                    xT[rs, ts].rearrange("(a p) m -> p a m", p=128))

        load_w(2)       # V weights first (g emission order is 2,0,1)
        load_x(0, 0)
        load_x(0, 1)
        load_w(0)
        load_w(1)
        if not zero_bias:
            nc.sync.dma_start(bqk_sb[:], bqk[:])
        load_x(1)
        load_x(2)
        nc.sync.dma_start(cos_sb[:], cosT[:])
        nc.sync.dma_start(sin_sb[:], sinS[:])
        load_x(3)
        nc.sync.dma_start(id2[:], dram["id2"][:])
        for c in range(4, 8):
            load_x(c)
        nc.sync.dma_start(wo_sb[:], woT[:])

        # ================= emission helpers =================
        evict_flip = [0]

        def qkv_unit(pool, c, g, tag="f"):
            """One projection group for one 512-token chunk: 8 accumulating
            matmuls + one psum->sbuf eviction (with cast to bf16)."""
            ts = slice(c * QCH, (c + 1) * QCH)
            ps = pool.tile([128, QCH], F32, tag=tag, name=f"psq{c}{g}")
            for ct in range(NCT):
                nc.tensor.matmul(ps[:], wsb[:, ct, g * 128:(g + 1) * 128],
                                 xsb[:, ct, ts],
                                 start=(ct == 0), stop=(ct == NCT - 1))
            dst = (qT, kT, vTf)[g]
            if zero_bias:
                nc.vector.tensor_copy(dst[:, ts], ps[:])
            else:
                if g == 2:
                    nc.scalar.activation(dst[:, ts], ps[:], AF.Copy)
                else:
                    nc.scalar.activation(dst[:, ts], ps[:], AF.Identity,
                                         bias=bqk_sb[:, g:g + 1])

        def vtrans_unit(pool, b, h, c, tag="f", tbufs=None):
            """Transpose V for the 4 k-tiles of one chunk into Vsb."""
            pst = pool.tile([128, 4 * HD], BF16, tag=tag, bufs=tbufs,
                            name=f"pst{b}{h}{c}")
            t0 = 4 * (c % 4)
            for j in range(4):
                tt = t0 + j
                nc.tensor.transpose(
                    pst[:, j * HD:(j + 1) * HD],
                    vTf[h * HD:(h + 1) * HD,
                        b * T + tt * 128: b * T + (tt + 1) * 128],
                    id2[h * HD:(h + 1) * HD, :])
            vov = Vsb[b][h][:].rearrange("p (t c) -> p t c", c=HD + 1)
            nc.vector.tensor_copy(
                vov[:, t0:t0 + 4, 0:HD],
                pst[:].rearrange("p (t c) -> p t c", c=HD))

        def rope_half(swpp, rtp, zt, b, nm):
            bs = slice(b * T, (b + 1) * T)
            swp = swpp.tile([128, T], BF16, tag="swp", name=f"swp{nm}")
            for h in range(2):
                o = h * 64
                nc.sync.dma_start(swp[o:o + 32, :], zt[o + 32:o + 64, bs])
                nc.sync.dma_start(swp[o + 32:o + 64, :], zt[o:o + 32, bs])
            tmp = rtp.tile([128, T], BF16, tag="rt", name=f"rt{nm}")
            nc.vector.tensor_mul(tmp[:], swp[:], sin_sb[:])
            nc.vector.tensor_mul(zt[:, bs], zt[:, bs], cos_sb[:])
            nc.vector.tensor_add(zt[:, bs], zt[:, bs], tmp[:])

        def proj_unit(pool, ybp, b, tt):
            """Out-projection for one 128-token tile: 2 matmuls + eviction +
            y DMA (bf16)."""
            for ncol in range(2):
                ps = pool.tile([128, QCH], F32, tag="f",
                               name=f"psy{b}{tt}{ncol}")
                nc.tensor.matmul(
                    ps[:], OT[b][:, tt * 128:(tt + 1) * 128],
                    wo_sb[:, ncol * QCH:(ncol + 1) * QCH],
                    start=True, stop=True)
                yb = ybp.tile([128, QCH], BF16, tag="yb",
                              name=f"yb{b}{tt}{ncol}")
                if evict_flip[0] % 2 == 0:
                    nc.vector.tensor_copy(yb[:], ps[:])
                else:
                    nc.scalar.activation(yb[:], ps[:], AF.Copy)
                evict_flip[0] += 1
                nc.sync.dma_start(
                    y[b * T + tt * 128: b * T + (tt + 1) * 128,
                      ncol * QCH:(ncol + 1) * QCH], yb[:])

        # ================= attention =================
        def attn_b(pools, b, fillers, post_qc=None):
            pss, pso, ptp, mbp, smol, bcp = pools
            ucount = [0]
            for qc in range(NQC):
                nk = 4 * (qc + 1) if mode == "causal" else NKT
                qs = slice(b * T + qc * QCH, b * T + (qc + 1) * QCH)
                psO = pso.tile([65, 2 * QCH], F32, tag="o", name=f"psO{b}{qc}")
                pts = {}

                def emit_pv(j, nk=nk, psO=psO, pts=pts):
                    st, sp = (j == 0), (j == nk - 1)
                    pt = pts.pop(j)
                    nc.tensor.matmul(
                        psO[:, 0:QCH],
                        Vsb[b][0][:, j * (HD + 1):(j + 1) * (HD + 1)],
                        pt[:, 0:QCH], start=st, stop=sp)
                    nc.tensor.matmul(
                        psO[:, QCH:2 * QCH],
                        Vsb[b][1][:, j * (HD + 1):(j + 1) * (HD + 1)],
                        pt[:, QCH:2 * QCH], start=st, stop=sp)

                for kt in range(nk):
                    ks = slice(b * T + kt * 128, b * T + (kt + 1) * 128)
                    psS = pss.tile([128, 2 * QCH], F32, tag="s",
                                   name=f"psS{b}{qc}{kt}")
                    nc.tensor.matmul(psS[:, 0:QCH], kT[0:64, ks],
                                     qT[0:64, qs], start=True, stop=True)
                    nc.tensor.matmul(psS[:, QCH:2 * QCH], kT[64:128, ks],
                                     qT[64:128, qs], start=True, stop=True)
                    pt = ptp.tile([128, 2 * QCH], BF16, tag="pt",
                                  name=f"pt{b}{qc}{kt}")
                    nc.scalar.activation(pt[:], psS[:], AF.Exp)
                    if mode == "causal" and kt >= 4 * qc:
                        base = qc * QCH - kt * 128
                        ptv = pt[:].rearrange("p (h q) -> p h q", q=QCH)
                        nc.gpsimd.affine_select(
                            out=ptv, in_=ptv, compare_op=ALU.is_ge,
                            fill=0.0, base=base, channel_multiplier=-1,
                            pattern=[[0, 2], [1, QCH]])
                    elif mode == "bias":
                        mt = mbp.tile([128, QCH], BF16, tag="mb",
                                      name=f"mt{b}{qc}{kt}")
                        nc.sync.dma_start(
                            mt[:], maskT[kt * 128:(kt + 1) * 128,
                                         qc * QCH:(qc + 1) * QCH])
                        nc.vector.tensor_mul(pt[:, 0:QCH], pt[:, 0:QCH],
                                             mt[:])
                        nc.vector.tensor_mul(pt[:, QCH:2 * QCH],
                                             pt[:, QCH:2 * QCH], mt[:])
                    pts[kt] = pt
                    # filler work to keep the PE warm while ScalarE exps
                    ucount[0] += 1
                    if fillers and ucount[0] >= 2:
                        fillers.popleft()()
                    if kt >= PIPE:
                        emit_pv(kt - PIPE)
                for j in range(max(0, nk - PIPE), nk):
                    emit_pv(j)

                # ---- normalize + evict: sum(exp) is row 64 of psO ----
                oqs = slice(qc * QCH, (qc + 1) * QCH)
                nm = f"{b}{qc}"
                rw = smol.tile([65, 2 * QCH], F32, tag="rw", name=f"rw{nm}")
                nc.vector.tensor_copy(rw[64:65, :], psO[64:65, :])
                bc = bcp.tile([128, 2 * QCH], F32, tag="bc", name=f"bc{nm}")
                if DIRECT_BCAST:
                    nc.gpsimd.partition_broadcast(bc[:], rw[64:65, :])
                else:
                    rz = smol.tile([1, 2 * QCH], F32, tag="rz", name=f"rz{nm}")
                    nc.sync.dma_start(rz[:], rw[64:65, :])
                    nc.gpsimd.partition_broadcast(bc[:], rz[:])
                otmp = bcp.tile([64, QCH], BF16, tag="otmp", name=f"ot{nm}")
                if USE_DIV:
                    nc.vector.tensor_tensor(OT[b][0:64, oqs],
                                            psO[0:64, 0:QCH],
                                            bc[0:64, 0:QCH], op=ALU.divide)
                    nc.vector.tensor_tensor(otmp[:], psO[0:64, QCH:2 * QCH],
                                            bc[0:64, QCH:2 * QCH],
                                            op=ALU.divide)
                else:
                    bcr = bcp.tile([128, 2 * QCH], F32, tag="bcr",
                                   name=f"bcr{nm}")
                    nc.vector.reciprocal_approx_fast(bcr[:], bc[:])
                    nc.vector.tensor_mul(OT[b][0:64, oqs], psO[0:64, 0:QCH],
                                         bcr[0:64, 0:QCH])
                    nc.vector.tensor_mul(otmp[:], psO[0:64, QCH:2 * QCH],
                                         bcr[0:64, QCH:2 * QCH])
                nc.sync.dma_start(OT[b][64:128, oqs], otmp[:])
                if post_qc is not None:
                    post_qc(qc)

        # ================= phases =================
        # Phase A: qkv + vtrans for batch 0 (dedicated psum pool), rope(b0)
        swpp = ctx.enter_context(tc.tile_pool(name="swp", bufs=2))
        rtp = ctx.enter_context(tc.tile_pool(name="rtmp", bufs=2))
        with tc.tile_pool(name="psA", bufs=6, space="PSUM") as psA:
            # vtrans for chunk c is emitted one chunk late so its input
            # (the V eviction, on VectorE) is never on the PE critical path
            for c in range(4):
                for g in (2, 0, 1):
                    qkv_unit(psA, c, g, tag="q")
                if c > 0:
                    for h in range(2):
                        vtrans_unit(psA, 0, h, c - 1, tag="t", tbufs=2)
            for h in range(2):
                vtrans_unit(psA, 0, h, 3, tag="t", tbufs=2)
            rope_half(swpp, rtp, qT, 0, "q0")
            rope_half(swpp, rtp, kT, 0, "k0")

        # Phase B: attention with fillers
        with tc.tile_pool(name="pss", bufs=2, space="PSUM") as pss, \
             tc.tile_pool(name="pso", bufs=1, space="PSUM") as pso, \
             tc.tile_pool(name="psf", bufs=2, space="PSUM") as psf, \
             tc.tile_pool(name="ptp", bufs=PIPE + 3) as ptp, \
             tc.tile_pool(name="mbp", bufs=4) as mbp, \
             tc.tile_pool(name="smol", bufs=2) as smol, \
             tc.tile_pool(name="bcp", bufs=2) as bcp, \
             tc.tile_pool(name="ybp", bufs=4) as ybp:
            pools = (pss, pso, ptp, mbp, smol, bcp)

            fillers = deque()
            for c in range(4, 8):
                for g in (2, 0, 1):
                    fillers.append(lambda c=c, g=g: qkv_unit(psf, c, g))
                for h in range(2):
                    fillers.append(
                        lambda h=h, c=c: vtrans_unit(psf, 1, h, c))
            fillers.append(lambda: rope_half(swpp, rtp, qT, 1, "q1"))
            fillers.append(lambda: rope_half(swpp, rtp, kT, 1, "k1"))

            attn_b(pools, 0, fillers)
            while fillers:
                fillers.popleft()()

            # proj(b0) interleaved into attn(b1); proj(b1) per-qc after norm
            fillers2 = deque()
            for tt in range(NKT):
                fillers2.append(lambda tt=tt: proj_unit(psf, ybp, 0, tt))

            def post_qc_b1(qc):
                for tt in range(4 * qc, 4 * qc + 4):
                    fillers2.append(
                        lambda tt=tt: proj_unit(psf, ybp, 1, tt))

            attn_b(pools, 1, fillers2, post_qc=post_qc_b1)
            while fillers2:
                fillers2.popleft()()


def _build_program(mode, zero_bias):
    key = (mode, zero_bias)
    if key in _PROG_CACHE:
        return _PROG_CACHE[key]
    nc = bacc.Bacc("TRN2", target_bir_lowering=False, debug=False,
                   num_devices=NCORES)
    dram = {
        "xT": nc.dram_tensor("xT", [C, TB], BF16, kind="ExternalInput").ap(),
        "wT": nc.dram_tensor("wT", [C, 384], BF16, kind="ExternalInput").ap(),
        "cosT": nc.dram_tensor("cosT", [128, T], BF16,
                               kind="ExternalInput").ap(),
        "sinS": nc.dram_tensor("sinS", [128, T], BF16,
                               kind="ExternalInput").ap(),
        "woT": nc.dram_tensor("woT", [128, C], BF16,
                              kind="ExternalInput").ap(),
        "id2": nc.dram_tensor("id2", [128, 64], BF16,
                              kind="ExternalInput").ap(),
        "y": nc.dram_tensor("y", [TB, C], BF16, kind="ExternalOutput").ap(),
    }
    if not zero_bias:
        dram["bqk"] = nc.dram_tensor("bqk", [128, 2], F32,
                                     kind="ExternalInput").ap()
    if mode == "bias":
        dram["maskT"] = nc.dram_tensor("maskT", [T, T], BF16,
                                       kind="ExternalInput").ap()
    with tile.TileContext(nc) as tc:
        _emit(tc, mode, zero_bias, dram)
    nc.compile()
    _PROG_CACHE[key] = (nc, dram)
    return nc, dram


def _rope_tables():
    inv_freq = 1.0 / (10000.0 ** (np.arange(0, HD, 2, dtype=np.float64) / HD))
    freqs = np.arange(T, dtype=np.float64)[:, None] * inv_freq[None, :]
    cos = np.concatenate([np.cos(freqs), np.cos(freqs)], axis=-1)  # [T, 64]
    sin = np.concatenate([np.sin(freqs), np.sin(freqs)], axis=-1)
    cE = cos[:, 0::2].T  # [32, T] rows i -> dim 2i
    cO = cos[:, 1::2].T
    sE = sin[:, 0::2].T
    sO = sin[:, 1::2].T
    cosT = np.concatenate([cE, cO, cE, cO], axis=0)
    sinS = np.concatenate([-sE, sO, -sE, sO], axis=0)
    return cosT, sinS


def _prepare(x, mask, Wqkv, bqkv, Wo, bo):
    """Host-side prep shared by kernel() and test harness profiling."""
    from ml_dtypes import bfloat16

    x = np.asarray(x, dtype=np.float32)
    mask = np.asarray(mask)
    Wqkv = np.asarray(Wqkv, dtype=np.float32)
    bqkv = np.asarray(bqkv, dtype=np.float32)
    Wo = np.asarray(Wo, dtype=np.float32)

    mb = mask.reshape(T, T)
    if np.array_equal(mb != 0, np.tril(np.ones((T, T), dtype=bool))):
        mode = "causal"
    elif np.all(mb != 0):
        mode = "dense"
    else:
        mode = "bias"
    zero_bias = bool(np.all(bqkv == 0.0))

    xTn = np.ascontiguousarray(x.reshape(TB, C).T).astype(bfloat16)
    cosT, sinS = _rope_tables()
    cosT = np.ascontiguousarray(cosT).astype(bfloat16)
    sinS = np.ascontiguousarray(sinS).astype(bfloat16)
    scale = 1.0 / np.sqrt(np.float64(HD))

    evens = np.arange(0, HD, 2)
    odds = evens + 1

    in_maps = []
    for c in range(NCORES):
        h0, h1 = 2 * c, 2 * c + 1
        qrows = np.concatenate([h0 * HD + evens, h0 * HD + odds,
                                h1 * HD + evens, h1 * HD + odds])
        krows = C + qrows
        vrows = np.concatenate([2 * C + h0 * HD + np.arange(HD),
                                2 * C + h1 * HD + np.arange(HD)])
        wq = Wqkv[qrows, :] * scale
        wk = Wqkv[krows, :]
        wv = Wqkv[vrows, :]
        wT = np.ascontiguousarray(
            np.concatenate([wq, wk, wv], axis=0).T).astype(bfloat16)
        woT = np.ascontiguousarray(Wo[:, 128 * c:128 * (c + 1)].T
                                   ).astype(bfloat16)
        id2 = np.concatenate([np.eye(64), np.eye(64)], axis=0).astype(bfloat16)
        im = {"xT": xTn, "wT": wT, "cosT": cosT, "sinS": sinS, "woT": woT,
              "id2": id2}
        if not zero_bias:
            bqk = np.stack([bqkv[qrows] * scale, bqkv[krows]], axis=1)
            im["bqk"] = np.ascontiguousarray(bqk, dtype=np.float32)
        if mode == "bias":
            im["maskT"] = np.ascontiguousarray(
                (mb != 0).astype(np.float32).T).astype(bfloat16)
        in_maps.append(im)
    return mode, zero_bias, in_maps


def kernel(x, mask, Wqkv, bqkv, Wo, bo):
    bqkv = np.asarray(bqkv, dtype=np.float32)
    Wo = np.asarray(Wo, dtype=np.float32)
    bo = np.asarray(bo, dtype=np.float32)

    mode, zero_bias, in_maps = _prepare(x, mask, Wqkv, bqkv, Wo, bo)
    nc, dram = _build_program(mode, zero_bias)

    res = run_bass_kernel_spmd(nc, in_maps, core_ids=list(range(NCORES)))
    y = np.zeros((TB, C), dtype=np.float32)
    for c in range(NCORES):
        y += np.asarray(res.results[c]["y"], dtype=np.float32)
    bv = bqkv[2 * C:3 * C]
    y += (bo + bv @ Wo.T)[None, :]
    return y.reshape(B, T, C)


# revision 33
# speedup vs baseline: 1.4248x; 1.0797x over previous
"""Trainium2 Bass kernel for CarlosSelfAttention (B=2, T=2048, C=1024, H=16).

Sharding: tensor-parallel over heads. 8 cores x 2 heads each.
Each core computes q/k/v projections for its 2 heads, RoPE, causal
attention, and a partial out-projection against its 128 columns of Wo.
The host sums the 8 partial outputs (the TP all-reduce) and adds the
output bias plus the (v-bias @ Wo.T) correction term.

v1 redesign vs baseline:
  - bf16 everywhere on-chip (PSUM stays fp32); x/weights/rope tables are
    host-cast to bf16; y partials returned as bf16 (summed on host in f32).
  - Single-pass pipelined emission: qkv(b0) -> [attn(b0) with qkv(b1) +
    rope(b1) + vtrans(b1) as PE/vector fillers] -> [attn(b1) with proj
    fillers] -> proj tail. Keeps TensorE dense so the HAM clock stays at
    2.4 GHz (baseline ran ~70% of the kernel at 1.2 GHz).
  - All of x preloaded into SBUF via 8 chunked DMAs (1 MB each).
  - PSUM plan (8 banks): psS 2x[128,1024] (4) + psO 1x[65,1024] (2) +
    filler/proj/transpose pool 2x[128,512] (2).
  - Softmax denominator via ones-column in the PV stationary (row 64 of
    psO); normalization = row-copy -> partition-shift DMA -> gpsimd
    broadcast -> DVE divide.
"""

import numpy as np

import concourse.bass as bass
import concourse.tile as tile
from concourse import bacc, mybir
from concourse.bass_utils import run_bass_kernel_spmd

F32 = mybir.dt.float32
F32R = mybir.dt.float32r
BF16 = mybir.dt.bfloat16
AF = mybir.ActivationFunctionType
ALU = mybir.AluOpType

B, T, C, H, HD = 2, 2048, 1024, 16, 64
NCORES = 8
TB = B * T          # 4096
QCH = 512           # q-chunk (moving dim)
NQC = T // QCH      # 4 q-chunks per batch
NKT = T // 128      # 16 k-tiles per batch
NCT = C // 128      # 8 contraction tiles
PIPE = 7            # exp->PV pipeline lag (in kt units); also hides the
                    # per-qc normalization latency behind the exp stream
USE_DIV = False     # DVE divide is not a valid TT op on trn2; use recip+mul

_PROG_CACHE: dict = {}


def _emit(tc, mode, zero_bias, dram):
    nc = tc.nc
    from contextlib import ExitStack
    from collections import deque

    xT, wT, cosT, sinS, woT, y = (
        dram["xT"], dram["wT"], dram["cosT"], dram["sinS"], dram["woT"],
        dram["y"])
    bqk = dram.get("bqk")
    maskT = dram.get("maskT")

    with ExitStack() as ctx:
        constp = ctx.enter_context(tc.tile_pool(name="const", bufs=1))
        pers = ctx.enter_context(tc.tile_pool(name="pers", bufs=1))

        # ---- persistent tiles ----
        xsb = pers.tile([128, NCT, TB], BF16)     # all of x, [c128, ct, tok]
        qT = pers.tile([128, TB], BF16)
        kT = pers.tile([128, TB], BF16)
        vTf = pers.tile([128, TB], BF16)
        Vsb = [[pers.tile([128, NKT * (HD + 1)], BF16, name=f"Vsb{b}{h}")
                for h in range(2)] for b in range(B)]
        OT = [pers.tile([128, T], BF16, name=f"OTb{b}") for b in range(B)]

        wsb = constp.tile([128, NCT, 384], BF16)
        cos_sb = constp.tile([128, T], BF16)
        sin_sb = constp.tile([128, T], BF16)
        wo_sb = constp.tile([128, C], BF16)
        id2 = constp.tile([128, 64], BF16)  # [eye(64); eye(64)] host-fed
        ones_sb = constp.tile([65, 128], F32)  # row 64 = K=1 broadcast lhsT
        nc.sync.dma_start(ones_sb[64:65, :].bitcast(F32R),
                          dram["ones1"][:].bitcast(F32R))
        if not zero_bias:
            bqk_sb = constp.tile([128, 2], F32)

        # ones column in each Vsb 65-block (the softmax-denominator row)
        for b in range(B):
            for h in range(2):
                vov = Vsb[b][h][:].rearrange("p (t c) -> p t c", c=HD + 1)
                nc.vector.memset(vov[:, :, HD:HD + 1], 1.0)

        # ---- DMA issue order tuned so the first qkv matmul starts ASAP ----
        def load_w(g):
            gs = slice(g * 128, (g + 1) * 128)
            nc.sync.dma_start(
                wsb[:, :, gs],
                wT[:, gs].rearrange("(a p) m -> p a m", p=128))

        def load_x(c, half=None):
            ts = slice(c * QCH, (c + 1) * QCH)
            if half is None:
                nc.sync.dma_start(
                    xsb[:, :, ts],
                    xT[:, ts].rearrange("(a p) m -> p a m", p=128))
            else:
                cs = slice(half * 4, half * 4 + 4)
                rs = slice(half * 512, half * 512 + 512)
                nc.sync.dma_start(
                    xsb[:, cs, ts],
                    xT[rs, ts].rearrange("(a p) m -> p a m", p=128))

        load_w(2)       # V weights first (g emission order is 2,0,1)
        load_x(0, 0)
        load_x(0, 1)
        load_w(0)
        load_w(1)
        if not zero_bias:
            nc.sync.dma_start(bqk_sb[:], bqk[:])
        load_x(1)
        load_x(2)
        nc.sync.dma_start(cos_sb[:], cosT[:])
        nc.sync.dma_start(sin_sb[:], sinS[:])
        load_x(3)
        nc.sync.dma_start(id2[:], dram["id2"][:])
        for c in range(4, 8):
            load_x(c)
        nc.sync.dma_start(wo_sb[:], woT[:])

        # ================= emission helpers =================
        evict_flip = [0]

        def qkv_unit(pool, c, g, tag="f"):
            """One projection group for one 512-token chunk: 8 accumulating
            matmuls + one psum->sbuf eviction (with cast to bf16)."""
            ts = slice(c * QCH, (c + 1) * QCH)
            ps = pool.tile([128, QCH], F32, tag=tag, name=f"psq{c}{g}")
            for ct in range(NCT):
                nc.tensor.matmul(ps[:], wsb[:, ct, g * 128:(g + 1) * 128],
                                 xsb[:, ct, ts],
                                 start=(ct == 0), stop=(ct == NCT - 1))
            dst = (qT, kT, vTf)[g]
            if zero_bias:
                nc.vector.tensor_copy(dst[:, ts], ps[:])
            else:
                if g == 2:
                    nc.scalar.activation(dst[:, ts], ps[:], AF.Copy)
                else:
                    nc.scalar.activation(dst[:, ts], ps[:], AF.Identity,
                                         bias=bqk_sb[:, g:g + 1])

        def vtrans_unit(pool, b, h, c, tag="f", tbufs=None):
            """Transpose V for the 4 k-tiles of one chunk into Vsb."""
            pst = pool.tile([128, 4 * HD], BF16, tag=tag, bufs=tbufs,
                            name=f"pst{b}{h}{c}")
            t0 = 4 * (c % 4)
            for j in range(4):
                tt = t0 + j
                nc.tensor.transpose(
                    pst[:, j * HD:(j + 1) * HD],
                    vTf[h * HD:(h + 1) * HD,
                        b * T + tt * 128: b * T + (tt + 1) * 128],
                    id2[h * HD:(h + 1) * HD, :])
            vov = Vsb[b][h][:].rearrange("p (t c) -> p t c", c=HD + 1)
            nc.vector.tensor_copy(
                vov[:, t0:t0 + 4, 0:HD],
                pst[:].rearrange("p (t c) -> p t c", c=HD))

        def rope_chunk(swpp, rtp, zt, c, nm):
            """RoPE one 512-token chunk in place (zt is qT or kT)."""
            ts = slice(c * QCH, (c + 1) * QCH)          # global token slice
            ps = slice((c % 4) * QCH, (c % 4 + 1) * QCH)  # position in batch
            swp = swpp.tile([128, QCH], BF16, tag="swp", name=f"swp{nm}")
            for h in range(2):
                o = h * 64
                nc.sync.dma_start(swp[o:o + 32, :], zt[o + 32:o + 64, ts])
                nc.sync.dma_start(swp[o + 32:o + 64, :], zt[o:o + 32, ts])
            tmp = rtp.tile([128, QCH], BF16, tag="rt", name=f"rt{nm}")
            nc.vector.tensor_mul(tmp[:], swp[:], sin_sb[:, ps])
            nc.vector.tensor_mul(zt[:, ts], zt[:, ts], cos_sb[:, ps])
            nc.vector.tensor_add(zt[:, ts], zt[:, ts], tmp[:])

        def proj_unit(pool, ybp, b, tt):
            """Out-projection for one 128-token tile: 2 matmuls + eviction +
            y DMA (bf16)."""
            for ncol in range(2):
                ps = pool.tile([128, QCH], F32, tag="f",
                               name=f"psy{b}{tt}{ncol}")
                nc.tensor.matmul(
                    ps[:], OT[b][:, tt * 128:(tt + 1) * 128],
                    wo_sb[:, ncol * QCH:(ncol + 1) * QCH],
                    start=True, stop=True)
                yb = ybp.tile([128, QCH], BF16, tag="yb",
                              name=f"yb{b}{tt}{ncol}")
                if evict_flip[0] % 2 == 0:
                    nc.vector.tensor_copy(yb[:], ps[:])
                else:
                    nc.scalar.activation(yb[:], ps[:], AF.Copy)
                evict_flip[0] += 1
                nc.sync.dma_start(
                    y[b * T + tt * 128: b * T + (tt + 1) * 128,
                      ncol * QCH:(ncol + 1) * QCH], yb[:])

        # ================= attention =================
        def attn_b(pools, b, fillers, post_qc=None):
            pss, pso, ptp, mbp, smol, bcp = pools
            ucount = [0]
            for qc in range(NQC):
                nk = 4 * (qc + 1) if mode == "causal" else NKT
                qs = slice(b * T + qc * QCH, b * T + (qc + 1) * QCH)
                psO = pso.tile([65, 2 * QCH], F32, tag="o", name=f"psO{b}{qc}")
                pts = {}

                def emit_pv(j, nk=nk, psO=psO, pts=pts):
                    st, sp = (j == 0), (j == nk - 1)
                    pt = pts.pop(j)
                    nc.tensor.matmul(
                        psO[:, 0:QCH],
                        Vsb[b][0][:, j * (HD + 1):(j + 1) * (HD + 1)],
                        pt[:, 0:QCH], start=st, stop=sp)
                    nc.tensor.matmul(
                        psO[:, QCH:2 * QCH],
                        Vsb[b][1][:, j * (HD + 1):(j + 1) * (HD + 1)],
                        pt[:, QCH:2 * QCH], start=st, stop=sp)

                for kt in range(nk):
                    ks = slice(b * T + kt * 128, b * T + (kt + 1) * 128)
                    psS = pss.tile([128, 2 * QCH], F32, tag="s",
                                   name=f"psS{b}{qc}{kt}")
                    nc.tensor.matmul(psS[:, 0:QCH], kT[0:64, ks],
                                     qT[0:64, qs], start=True, stop=True)
                    nc.tensor.matmul(psS[:, QCH:2 * QCH], kT[64:128, ks],
                                     qT[64:128, qs], start=True, stop=True)
                    pt = ptp.tile([128, 2 * QCH], BF16, tag="pt",
                                  name=f"pt{b}{qc}{kt}")
                    nc.scalar.activation(pt[:], psS[:], AF.Exp)
                    if mode == "causal" and kt >= 4 * qc:
                        base = qc * QCH - kt * 128
                        ptv = pt[:].rearrange("p (h q) -> p h q", q=QCH)
                        nc.gpsimd.affine_select(
                            out=ptv, in_=ptv, compare_op=ALU.is_ge,
                            fill=0.0, base=base, channel_multiplier=-1,
                            pattern=[[0, 2], [1, QCH]])
                    elif mode == "bias":
                        mt = mbp.tile([128, QCH], BF16, tag="mb",
                                      name=f"mt{b}{qc}{kt}")
                        nc.sync.dma_start(
                            mt[:], maskT[kt * 128:(kt + 1) * 128,
                                         qc * QCH:(qc + 1) * QCH])
                        nc.vector.tensor_mul(pt[:, 0:QCH], pt[:, 0:QCH],
                                             mt[:])
                        nc.vector.tensor_mul(pt[:, QCH:2 * QCH],
                                             pt[:, QCH:2 * QCH], mt[:])
                    pts[kt] = pt
                    # filler work to keep the PE warm while ScalarE exps
                    ucount[0] += 1
                    if fillers and ucount[0] >= 2:
                        fillers.popleft()()
                    if kt >= PIPE:
                        emit_pv(kt - PIPE)
                for j in range(max(0, nk - PIPE), nk):
                    emit_pv(j)

                # ---- normalize + evict: sum(exp) is row 64 of psO ----
                oqs = slice(qc * QCH, (qc + 1) * QCH)
                nm = f"{b}{qc}"
                rw = smol.tile([65, 2 * QCH], F32R, tag="rw", name=f"rw{nm}")
                nc.vector.tensor_copy(rw[64:65, :], psO[64:65, :])
                # broadcast den across partitions via a K=1 ones-matmul
                bc = pss.tile([128, 2 * QCH], F32, tag="s", name=f"bc{nm}")
                for half in range(2):
                    hs = slice(half * QCH, (half + 1) * QCH)
                    nc.tensor.matmul(bc[:, hs],
                                     ones_sb[64:65, :].bitcast(F32R),
                                     rw[64:65, hs],
                                     start=True, stop=True)
                bcr = bcp.tile([128, 2 * QCH], F32, tag="bcr", name=f"bcr{nm}")
                nc.vector.reciprocal_approx_fast(bcr[:], bc[:])
                otmp = bcp.tile([64, QCH], BF16, tag="otmp", name=f"ot{nm}")
                nc.vector.tensor_mul(OT[b][0:64, oqs], psO[0:64, 0:QCH],
                                     bcr[0:64, 0:QCH])
                nc.vector.tensor_mul(otmp[:], psO[0:64, QCH:2 * QCH],
                                     bcr[0:64, QCH:2 * QCH])
                nc.sync.dma_start(OT[b][64:128, oqs], otmp[:])
                if post_qc is not None:
                    post_qc(qc)

        # ================= phases =================
        # Phase A: qkv + vtrans for batch 0 (dedicated psum pool), rope(b0)
        swpp = ctx.enter_context(tc.tile_pool(name="swp", bufs=2))
        rtp = ctx.enter_context(tc.tile_pool(name="rtmp", bufs=2))
        with tc.tile_pool(name="psA", bufs=6, space="PSUM") as psA:
            # vtrans for chunk c is emitted one chunk late so its input
            # (the V eviction, on VectorE) is never on the PE critical path
            for c in range(4):
                qkv_unit(psA, c, 2, tag="q")
                qkv_unit(psA, c, 0, tag="q")
                rope_chunk(swpp, rtp, qT, c, f"q{c}")
                qkv_unit(psA, c, 1, tag="q")
                rope_chunk(swpp, rtp, kT, c, f"k{c}")
                if c > 0:
                    for h in range(2):
                        vtrans_unit(psA, 0, h, c - 1, tag="t", tbufs=2)
            for h in range(2):
                vtrans_unit(psA, 0, h, 3, tag="t", tbufs=2)

        # Phase B: attention with fillers
        with tc.tile_pool(name="pss", bufs=2, space="PSUM") as pss, \
             tc.tile_pool(name="pso", bufs=1, space="PSUM") as pso, \
             tc.tile_pool(name="psf", bufs=2, space="PSUM") as psf, \
             tc.tile_pool(name="ptp", bufs=PIPE + 3) as ptp, \
             tc.tile_pool(name="mbp", bufs=4) as mbp, \
             tc.tile_pool(name="smol", bufs=2) as smol, \
             tc.tile_pool(name="bcp", bufs=2) as bcp, \
             tc.tile_pool(name="ybp", bufs=4) as ybp:
            pools = (pss, pso, ptp, mbp, smol, bcp)

            fillers = deque()
            for c in range(4, 8):
                fillers.append(lambda c=c: qkv_unit(psf, c, 2))
                fillers.append(lambda c=c: qkv_unit(psf, c, 0))
                fillers.append(
                    lambda c=c: rope_chunk(swpp, rtp, qT, c, f"q{c}"))
                fillers.append(lambda c=c: qkv_unit(psf, c, 1))
                fillers.append(
                    lambda c=c: rope_chunk(swpp, rtp, kT, c, f"k{c}"))
                for h in range(2):
                    fillers.append(
                        lambda h=h, c=c: vtrans_unit(psf, 1, h, c))

            attn_b(pools, 0, fillers)
            while fillers:
                fillers.popleft()()

            # proj(b0) interleaved into attn(b1); proj(b1) per-qc after norm
            fillers2 = deque()
            for tt in range(NKT):
                fillers2.append(lambda tt=tt: proj_unit(psf, ybp, 0, tt))

            def post_qc_b1(qc):
                for tt in range(4 * qc, 4 * qc + 4):
                    fillers2.append(
                        lambda tt=tt: proj_unit(psf, ybp, 1, tt))

            attn_b(pools, 1, fillers2, post_qc=post_qc_b1)
            while fillers2:
                fillers2.popleft()()


def _build_program(mode, zero_bias):
    key = (mode, zero_bias)
    if key in _PROG_CACHE:
        return _PROG_CACHE[key]
    nc = bacc.Bacc("TRN2", target_bir_lowering=False, debug=False,
                   num_devices=NCORES)
    dram = {
        "xT": nc.dram_tensor("xT", [C, TB], BF16, kind="ExternalInput").ap(),
        "wT": nc.dram_tensor("wT", [C, 384], BF16, kind="ExternalInput").ap(),
        "cosT": nc.dram_tensor("cosT", [128, T], BF16,
                               kind="ExternalInput").ap(),
        "sinS": nc.dram_tensor("sinS", [128, T], BF16,
                               kind="ExternalInput").ap(),
        "woT": nc.dram_tensor("woT", [128, C], BF16,
                              kind="ExternalInput").ap(),
        "id2": nc.dram_tensor("id2", [128, 64], BF16,
                              kind="ExternalInput").ap(),
        "ones1": nc.dram_tensor("ones1", [1, 128], F32,
                                kind="ExternalInput").ap(),
        "y": nc.dram_tensor("y", [TB, C], BF16, kind="ExternalOutput").ap(),
    }
    if not zero_bias:
        dram["bqk"] = nc.dram_tensor("bqk", [128, 2], F32,
                                     kind="ExternalInput").ap()
    if mode == "bias":
        dram["maskT"] = nc.dram_tensor("maskT", [T, T], BF16,
                                       kind="ExternalInput").ap()
    with tile.TileContext(nc) as tc:
        _emit(tc, mode, zero_bias, dram)
    nc.compile()
    _PROG_CACHE[key] = (nc, dram)
    return nc, dram


def _rope_tables():
    inv_freq = 1.0 / (10000.0 ** (np.arange(0, HD, 2, dtype=np.float64) / HD))
    freqs = np.arange(T, dtype=np.float64)[:, None] * inv_freq[None, :]
    cos = np.concatenate([np.cos(freqs), np.cos(freqs)], axis=-1)  # [T, 64]
    sin = np.concatenate([np.sin(freqs), np.sin(freqs)], axis=-1)
    cE = cos[:, 0::2].T  # [32, T] rows i -> dim 2i
    cO = cos[:, 1::2].T
    sE = sin[:, 0::2].T
    sO = sin[:, 1::2].T
    cosT = np.concatenate([cE, cO, cE, cO], axis=0)
    sinS = np.concatenate([-sE, sO, -sE, sO], axis=0)
    return cosT, sinS


def _prepare(x, mask, Wqkv, bqkv, Wo, bo):
    """Host-side prep shared by kernel() and test harness profiling."""
    from ml_dtypes import bfloat16

    x = np.asarray(x, dtype=np.float32)
    mask = np.asarray(mask)
    Wqkv = np.asarray(Wqkv, dtype=np.float32)
    bqkv = np.asarray(bqkv, dtype=np.float32)
    Wo = np.asarray(Wo, dtype=np.float32)

    mb = mask.reshape(T, T)
    if np.array_equal(mb != 0, np.tril(np.ones((T, T), dtype=bool))):
        mode = "causal"
    elif np.all(mb != 0):
        mode = "dense"
    else:
        mode = "bias"
    zero_bias = bool(np.all(bqkv == 0.0))

    xTn = np.ascontiguousarray(x.reshape(TB, C).T).astype(bfloat16)
    cosT, sinS = _rope_tables()
    cosT = np.ascontiguousarray(cosT).astype(bfloat16)
    sinS = np.ascontiguousarray(sinS).astype(bfloat16)
    scale = 1.0 / np.sqrt(np.float64(HD))

    evens = np.arange(0, HD, 2)
    odds = evens + 1

    in_maps = []
    for c in range(NCORES):
        h0, h1 = 2 * c, 2 * c + 1
        qrows = np.concatenate([h0 * HD + evens, h0 * HD + odds,
                                h1 * HD + evens, h1 * HD + odds])
        krows = C + qrows
        vrows = np.concatenate([2 * C + h0 * HD + np.arange(HD),
                                2 * C + h1 * HD + np.arange(HD)])
        wq = Wqkv[qrows, :] * scale
        wk = Wqkv[krows, :]
        wv = Wqkv[vrows, :]
        wT = np.ascontiguousarray(
            np.concatenate([wq, wk, wv], axis=0).T).astype(bfloat16)
        woT = np.ascontiguousarray(Wo[:, 128 * c:128 * (c + 1)].T
                                   ).astype(bfloat16)
        id2 = np.concatenate([np.eye(64), np.eye(64)], axis=0).astype(bfloat16)
        im = {"xT": xTn, "wT": wT, "cosT": cosT, "sinS": sinS, "woT": woT,
              "id2": id2, "ones1": np.ones((1, 128), dtype=np.float32)}
        if not zero_bias:
            bqk = np.stack([bqkv[qrows] * scale, bqkv[krows]], axis=1)
            im["bqk"] = np.ascontiguousarray(bqk, dtype=np.float32)
        if mode == "bias":
            im["maskT"] = np.ascontiguousarray(
                (mb != 0).astype(np.float32).T).astype(bfloat16)
        in_maps.append(im)
    return mode, zero_bias, in_maps


def kernel(x, mask, Wqkv, bqkv, Wo, bo):
    bqkv = np.asarray(bqkv, dtype=np.float32)
    Wo = np.asarray(Wo, dtype=np.float32)
    bo = np.asarray(bo, dtype=np.float32)

    mode, zero_bias, in_maps = _prepare(x, mask, Wqkv, bqkv, Wo, bo)
    nc, dram = _build_program(mode, zero_bias)

    res = run_bass_kernel_spmd(nc, in_maps, core_ids=list(range(NCORES)))
    y = np.zeros((TB, C), dtype=np.float32)
    for c in range(NCORES):
        y += np.asarray(res.results[c]["y"], dtype=np.float32)
    bv = bqkv[2 * C:3 * C]
    y += (bo + bv @ Wo.T)[None, :]
    return y.reshape(B, T, C)
